# revision 1
# baseline (speedup 1.0000x reference)
"""Trainium2 Bass kernel for LoRA attention prefill (B=4, S=1024, D=4096, H=32).

Sharding: tensor-parallel over heads. Each of the 8 cores computes 4 heads
(512 of the 4096 q/k/v features, column-shard of wq/wk/wv + LoRA B) and a
row-shard of wo, producing a full-shape [T, D] partial output; partials are
summed on the host.

Device layout choices:
  - All matmuls in bf16 with fp32 PSUM accumulation.
  - Activations kept feature-on-partition ("transposed"): xT [D, T] feeds
    Q/K projections directly as PE operands; V is computed token-on-partition
    so it can serve as the PV-matmul stationary operand without transposes.
  - RoPE handled by host-permuting wq/wk rows so each head's real/imag halves
    land in lane-aligned 64-partition blocks of separate psum tiles; scores
    then contract each head with two K=64 matmuls.
  - Attention in "scoresT" layout (keys on partitions): softmax denominator
    via a ones-column matmul on PE, probs feed the PV matmul untransposed,
    normalization applied to the attention output via a PE broadcast of the
    reciprocal sums.
"""
import sys
from contextlib import ExitStack

sys.path.insert(0, "/opt/trn_rl_repo")

import numpy as np
import ml_dtypes

import concourse.bass as bass
import concourse.mybir as mybir
import concourse.tile as tile
from concourse import bacc
from concourse.bass_utils import run_bass_kernel_spmd
from concourse.tile import TileContext

B, S, D = 4, 1024, 4096
H, HD = 32, 128
R = 16
LORA_SCALE = 2.0
N_CORES = 8
HPC = H // N_CORES            # heads per core
FPC = HPC * HD                # features per core = 512
T = B * S                     # 4096 tokens
TT = 256                      # stage-A T-tile (tokens)
NTT = S // TT                 # T-tiles per batch = 4
SCALE = float(1.0 / np.sqrt(HD))
BF = mybir.dt.bfloat16
F32 = mybir.dt.float32


def _bf(a):
    return np.ascontiguousarray(np.asarray(a, np.float32).astype(ml_dtypes.bfloat16))


def _core_perm(c):
    hs = [HPC * c + i for i in range(HPC)]
    ev = np.arange(0, HD, 2)
    od = np.arange(1, HD, 2)
    out = []
    for pair in (0, 1):
        h0, h1 = hs[2 * pair], hs[2 * pair + 1]
        out.append(h0 * HD + ev)
        out.append(h1 * HD + ev)
        out.append(h0 * HD + od)
        out.append(h1 * HD + od)
    return np.concatenate(out)


def _check_causal(mask):
    iu = np.triu_indices(S, k=1)
    il = np.tril_indices(S, k=0)
    return bool((mask[iu] <= -1e8).all() and (mask[il] == 0).all())


def _host_prep(x, wq_w, wq_a, wq_b, wk_w, wv_w, wv_a, wv_b, wo_w,
               freqs_cos, freqs_sin, mask):
    x2 = np.asarray(x, np.float32).reshape(T, D)
    xT = _bf(x2.T)
    waT = np.zeros((D, 48), np.float32)
    waT[:, 0:16] = np.asarray(wq_a, np.float32).T
    waT[:, 32:48] = np.asarray(wv_a, np.float32).T
    waT = _bf(waT)

    cosT = np.asarray(freqs_cos, np.float32).T
    sinT = np.asarray(freqs_sin, np.float32).T
    cc = np.ascontiguousarray(np.tile(cosT, (2, B)).astype(np.float32))
    ss = np.ascontiguousarray(np.tile(sinT, (2, B)).astype(np.float32))

    mask = np.asarray(mask, np.float32)
    causal = _check_causal(mask)
    mT = mask.T * np.float32(np.sqrt(HD))
    if causal:
        # diag-band blocks are identical for both query halves; verify
        maskp = np.zeros((4, 128, 512), np.float32)
        for j in range(4):
            maskp[j] = mT[j * 128:(j + 1) * 128, 0:512]
            if not np.array_equal(
                    maskp[j], mT[(4 + j) * 128:(5 + j) * 128, 512:1024]):
                causal = False
                break
    if not causal:
        maskp = np.zeros((8, 128, 2, 512), np.float32)
        for qh in range(2):
            for j in range(8):
                maskp[j, :, qh, :] = mT[j * 128:(j + 1) * 128,
                                        qh * 512:(qh + 1) * 512]

    shared = dict(xT=xT, waT=waT, cc=cc, ss=ss, maskp=maskp)
    cores = []
    for c in range(N_CORES):
        perm = _core_perm(c)
        sl = slice(c * FPC, (c + 1) * FPC)
        cores.append(dict(
            wqT=_bf(np.asarray(wq_w, np.float32)[perm, :].T),
            wkT=_bf(np.asarray(wk_w, np.float32)[perm, :].T),
            wvT=_bf(np.asarray(wv_w, np.float32)[sl, :].T),
            wqbT=_bf((np.asarray(wq_b, np.float32)[perm, :] * LORA_SCALE).T),
            wvbT=_bf((np.asarray(wv_b, np.float32)[sl, :] * LORA_SCALE).T),
            woT=_bf(np.asarray(wo_w, np.float32)[:, sl].T),
        ))
    return shared, cores, causal


def _build_program(causal):
    nc = bacc.Bacc("TRN2", num_devices=N_CORES)
    dt = mybir.dt
    nkb = 4 if causal else 8

    xT = nc.dram_tensor("xT", [D, T], BF, kind="ExternalInput").ap()
    wqT = nc.dram_tensor("wqT", [D, FPC], BF, kind="ExternalInput").ap()
    wkT = nc.dram_tensor("wkT", [D, FPC], BF, kind="ExternalInput").ap()
    wvT = nc.dram_tensor("wvT", [D, FPC], BF, kind="ExternalInput").ap()
    waT = nc.dram_tensor("waT", [D, 48], BF, kind="ExternalInput").ap()
    wqbT = nc.dram_tensor("wqbT", [R, FPC], BF, kind="ExternalInput").ap()
    wvbT = nc.dram_tensor("wvbT", [R, FPC], BF, kind="ExternalInput").ap()
    woT = nc.dram_tensor("woT", [FPC, D], BF, kind="ExternalInput").ap()
    cc = nc.dram_tensor("cc", [128, T], F32, kind="ExternalInput").ap()
    ss = nc.dram_tensor("ss", [128, T], F32, kind="ExternalInput").ap()
    mshape = [4, 128, 512] if causal else [8, 128, 2, 512]
    maskp = nc.dram_tensor("maskp", mshape, F32, kind="ExternalInput").ap()
    y = nc.dram_tensor("y", [T, D], F32, kind="ExternalOutput").ap()

    with TileContext(nc) as tc, ExitStack() as ctx:
        wpool = ctx.enter_context(tc.tile_pool(name="wpool", bufs=1))
        xpool = ctx.enter_context(tc.tile_pool(name="xpool", bufs=2))
        ccp = ctx.enter_context(tc.tile_pool(name="ccp", bufs=1))
        qkvp = ctx.enter_context(tc.tile_pool(name="qkvp", bufs=1))
        xap = ctx.enter_context(tc.tile_pool(name="xap", bufs=2))
        expp = ctx.enter_context(tc.tile_pool(name="expp", bufs=3))
        otp = ctx.enter_context(tc.tile_pool(name="otp", bufs=1))
        outp = ctx.enter_context(tc.tile_pool(name="outp", bufs=3))
        tmpp = ctx.enter_context(tc.tile_pool(name="tmpp", bufs=6))
        stp = ctx.enter_context(tc.tile_pool(name="stp", bufs=4))
        sump = ctx.enter_context(tc.tile_pool(name="sump", bufs=1))
        wop = ctx.enter_context(tc.tile_pool(name="wop", bufs=2))
        psa = ctx.enter_context(tc.tile_pool(name="psac", bufs=3, space="PSUM"))
        psc = psa
        psb = ctx.enter_context(tc.tile_pool(name="psb", bufs=5, space="PSUM"))

        # resident weights
        wq_sb = wpool.tile([128, 32, FPC], BF, tag="wq")
        nc.sync.dma_start(wq_sb[:], wqT.rearrange("(o p) f -> p o f", p=128))
        wk_sb = wpool.tile([128, 32, FPC], BF, tag="wk")
        nc.sync.dma_start(wk_sb[:], wkT.rearrange("(o p) f -> p o f", p=128))
        wv_sb = wpool.tile([128, 32, FPC], BF, tag="wv")
        nc.sync.dma_start(wv_sb[:], wvT.rearrange("(o p) f -> p o f", p=128))
        wa_sb = wpool.tile([128, 32, 48], BF, tag="wa")
        nc.sync.dma_start(wa_sb[:], waT.rearrange("(o p) f -> p o f", p=128))
        wqb_sb = wpool.tile([R, FPC], BF, tag="wqb")
        nc.sync.dma_start(wqb_sb[:], wqbT[:])
        # parked at partitions 32-47 so the V-lora matmul's lhsT (xa rows
        # 32-47) and rhs share a base partition, as the PE requires
        wvb_sb = wpool.tile([48, FPC], BF, tag="wvb")
        nc.sync.dma_start(wvb_sb[32:48, :], wvbT[:])
        if causal:
            mask_sb = wpool.tile([128, 4, 512], F32, tag="mask")
            nc.sync.dma_start(mask_sb[:], maskp.rearrange("j p n -> p j n"))
        else:
            mask_sb = wpool.tile([128, 8, 2, 512], F32, tag="mask")
            nc.sync.dma_start(mask_sb[:],
                              maskp.rearrange("j p q n -> p j q n"))
        ones_col = wpool.tile([128, 1], BF, tag="onec")
        nc.gpsimd.memset(ones_col[:], 1.0)
        ones_row = wpool.tile([1, 128], F32, tag="oner")
        nc.gpsimd.memset(ones_row[:], 1.0)

        for b in range(B):
            Q_sb = qkvp.tile([128, 4, S], BF, tag="Qsb")
            K_sb = qkvp.tile([128, 4, S], BF, tag="Ksb")
            V_sb = qkvp.tile([128, 8, FPC], BF, tag="Vsb")
            OT_sb = otp.tile([128, 4, S], BF, tag="OT")

            def attn_half(qh):
                q0 = qh * 512
                kbs = list(range(0, qh * 4 + 4)) if causal else list(range(8))
                last = len(kbs) - 1
                for l in range(HPC):
                    ps_ot = psb.tile([128, 512], F32, tag="psb")
                    ps_sum = psb.tile([128, 512], F32, tag="psb")
                    for j, kb in enumerate(kbs):
                        k0 = kb * 128
                        ps_sc = psb.tile([128, 512], F32, tag="psb")
                        nc.tensor.matmul(
                            ps_sc[:], K_sb[:, l, k0:k0 + 128],
                            Q_sb[:, l, q0:q0 + 512], start=True, stop=True)
                        if causal:
                            if kb >= qh * 4:
                                nc.vector.tensor_add(
                                    ps_sc[:], ps_sc[:],
                                    mask_sb[:, kb - qh * 4, :])
                        else:
                            nc.vector.tensor_add(
                                ps_sc[:], ps_sc[:], mask_sb[:, kb, qh, :])
                        e_sb = expp.tile([128, 512], BF, tag="e")
                        nc.scalar.activation(
                            e_sb[:], ps_sc[:],
                            mybir.ActivationFunctionType.Exp, scale=SCALE)
                        nc.tensor.matmul(ps_sum[0:1, :], ones_col[:], e_sb[:],
                                         start=(j == 0), stop=(j == last))
                        nc.tensor.matmul(
                            ps_ot[:], V_sb[:, kb, l * 128:(l + 1) * 128],
                            e_sb[:], start=(j == 0), stop=(j == last))
                    # normalization: keep the slow reciprocal off PSUM so the
                    # next head's matmuls aren't starved of banks
                    sum_sb = sump.tile([1, 512], F32, tag="sum")
                    nc.scalar.copy(sum_sb[:], ps_sum[0:1, :])
                    rec1_sb = sump.tile([1, 512], F32, tag="rec1")
                    nc.vector.reciprocal(rec1_sb[:], sum_sb[:])
                    ps_bc = psb.tile([128, 512], F32, tag="psb")
                    nc.tensor.matmul(ps_bc[:], ones_row[:], rec1_sb[:],
                                     start=True, stop=True)
                    rec_sb = outp.tile([128, 512], F32, tag="o")
                    nc.vector.tensor_copy(rec_sb[:], ps_bc[:])
                    nc.vector.tensor_mul(OT_sb[:, l, q0:q0 + 512], ps_ot[:],
                                         rec_sb[:])

            # ---- stage A: projections + RoPE (attention qh=0 interleaved) --
            for tt in range(NTT):
                t0 = b * S + tt * TT
                x_sb = xpool.tile([128, 32, TT], BF, tag="x")
                nc.sync.dma_start(
                    x_sb[:],
                    xT.rearrange("(o p) t -> p o t", p=128)[:, :, t0:t0 + TT])
                cc_sb = ccp.tile([128, TT], F32, tag="cc")
                nc.sync.dma_start(cc_sb[:], cc[:, t0:t0 + TT])
                ss_sb = ccp.tile([128, TT], F32, tag="ss")
                nc.sync.dma_start(ss_sb[:], ss[:, t0:t0 + TT])

                # lora A: xa[48, TT]
                ps_xa = psa.tile([128, 512], F32, tag="psa")
                for d in range(32):
                    nc.tensor.matmul(ps_xa[0:48, 0:TT], wa_sb[:, d, :],
                                     x_sb[:, d, :], start=(d == 0),
                                     stop=(d == 31))
                xa_sb = xap.tile([48, TT], BF, tag="xa")
                nc.scalar.copy(xa_sb[:], ps_xa[0:48, 0:TT])

                # Q and K, RoPE'd into Q_sb/K_sb
                for dst_sb, w_sb, lora in ((Q_sb, wq_sb, True),
                                           (K_sb, wk_sb, False)):
                    for pair in range(2):
                        ps_pair = []
                        for ri in range(2):
                            f0 = pair * 256 + ri * 128
                            ps = psa.tile([128, 512], F32, tag="psa")
                            for d in range(32):
                                nc.tensor.matmul(
                                    ps[:, 0:TT], w_sb[:, d, f0:f0 + 128],
                                    x_sb[:, d, :], start=(d == 0),
                                    stop=(d == 31 and not lora))
                            if lora:
                                nc.tensor.matmul(
                                    ps[:, 0:TT], wqb_sb[:, f0:f0 + 128],
                                    xa_sb[0:16, :], start=False, stop=True)
                            ps_pair.append(ps)
                        ps_r, ps_i = ps_pair
                        toff = tt * TT
                        t1 = tmpp.tile([128, TT], F32, tag="t")
                        nc.vector.tensor_mul(t1[:], ps_r[:, 0:TT], cc_sb[:])
                        t2 = tmpp.tile([128, TT], F32, tag="t")
                        nc.vector.tensor_mul(t2[:], ps_i[:, 0:TT], ss_sb[:])
                        st_r = stp.tile([128, TT], BF, tag="st")
                        nc.vector.tensor_tensor(
                            st_r[:], t1[:], t2[:], mybir.AluOpType.subtract)
                        t3 = tmpp.tile([128, TT], F32, tag="t")
                        nc.vector.tensor_mul(t3[:], ps_r[:, 0:TT], ss_sb[:])
                        t4 = tmpp.tile([128, TT], F32, tag="t")
                        nc.vector.tensor_mul(t4[:], ps_i[:, 0:TT], cc_sb[:])
                        st_i = stp.tile([128, TT], BF, tag="st")
                        nc.vector.tensor_tensor(
                            st_i[:], t3[:], t4[:], mybir.AluOpType.add)
                        # shuffle into head-contiguous blocks: head h of this
                        # pair = [r half; i half] on partitions [0:64|64:128]
                        for hh in range(2):
                            h_loc = 2 * pair + hh
                            nc.sync.dma_start(
                                dst_sb[0:64, h_loc, toff:toff + TT],
                                st_r[hh * 64:(hh + 1) * 64, :])
                            nc.sync.dma_start(
                                dst_sb[64:128, h_loc, toff:toff + TT],
                                st_i[hh * 64:(hh + 1) * 64, :])

                # V natural: per 128-token block
                for v in range(TT // 128):
                    tb = tt * (TT // 128) + v
                    ps_v = psa.tile([128, 512], F32, tag="psa")
                    for d in range(32):
                        nc.tensor.matmul(
                            ps_v[:], x_sb[:, d, v * 128:(v + 1) * 128],
                            wv_sb[:, d, :], start=(d == 0), stop=False)
                    nc.tensor.matmul(
                        ps_v[:], xa_sb[32:48, v * 128:(v + 1) * 128],
                        wvb_sb[32:48, :], start=False, stop=True)
                    nc.scalar.copy(V_sb[:, tb, :], ps_v[:])

                if tt == 1:
                    attn_half(0)
            attn_half(1)

            # ---- stage C: wo ----
            for nt in range(8):
                wo_sb = wop.tile([128, 4, 512], BF, tag="wo")
                nc.sync.dma_start(
                    wo_sb[:],
                    woT.rearrange("(o p) n -> p o n",
                                  p=128)[:, :, nt * 512:(nt + 1) * 512])
                for tb in range(8):
                    ps_o = psc.tile([128, 512], F32, tag="psa")
                    for k in range(4):
                        nc.tensor.matmul(
                            ps_o[:], OT_sb[:, k, tb * 128:(tb + 1) * 128],
                            wo_sb[:, k, :], start=(k == 0), stop=(k == 3))
                    o_sb = outp.tile([128, 512], F32, tag="o")
                    nc.scalar.copy(o_sb[:], ps_o[:])
                    nc.sync.dma_start(
                        y[b * S + tb * 128:b * S + (tb + 1) * 128,
                          nt * 512:(nt + 1) * 512], o_sb[:])

    nc.compile()
    return nc


_CACHE = {}


def _get_program(causal):
    if causal not in _CACHE:
        _CACHE[causal] = _build_program(causal)
    return _CACHE[causal]


def kernel(x, wq_w, wq_a, wq_b, wk_w, wv_w, wv_a, wv_b, wo_w,
           freqs_cos, freqs_sin, mask, start_pos=0, _trace=False):
    assert int(np.asarray(start_pos)) == 0
    shared, cores, causal = _host_prep(
        x, wq_w, wq_a, wq_b, wk_w, wv_w, wv_a, wv_b, wo_w,
        freqs_cos, freqs_sin, mask)
    nc = _get_program(causal)
    in_maps = []
    for c in range(N_CORES):
        m = dict(xT=shared["xT"], waT=shared["waT"], cc=shared["cc"],
                 ss=shared["ss"], maskp=shared["maskp"])
        m.update(cores[c])
        in_maps.append(m)
    res = run_bass_kernel_spmd(nc, in_maps, list(range(N_CORES)),
                               trace=_trace)
    kernel._last_results = res
    acc = np.zeros((T, D), np.float32)
    for c in range(N_CORES):
        acc += np.asarray(res.results[c]["y"], np.float32)
    out = acc.reshape(B, S, D)
    return out.astype(np.asarray(x).dtype, copy=False)



# revision 10
# speedup vs baseline: 1.2775x; 1.2775x over previous
"""Trainium2 Bass kernel for LoRA attention prefill (B=4, S=1024, D=4096, H=32).

Sharding: tensor-parallel over heads. Each of the 8 cores computes 4 heads
(512 of the 4096 q/k/v features, column-shard of wq/wk/wv) and a row-shard
of wo, producing a full-shape [T, D] partial output in bf16; partials are
summed on the host.

Device layout choices:
  - LoRA folded into wq/wv on the host (rank-16 update), so the device only
    runs dense projections.
  - All matmuls in bf16 with fp32 PSUM accumulation.
  - Activations kept feature-on-partition: xT [D, T] feeds Q/K projections
    directly as PE operands; V is computed token-on-partition so it serves
    as the PV-matmul stationary operand without transposes.
  - RoPE: wq/wk rows host-permuted so each head pair's even (real) and odd
    (imag) feature halves come out of separate psum chains lane-aligned with
    cos/sin tiles; the rotated results are stored in separate Qr/Qi (Kr/Ki)
    tiles with NO partition regrouping. Scores for head h contract as two
    K=64 matmuls (real + imag) on disjoint PE row tiles; the two heads of a
    pair are interleaved so the row tiles can overlap on the array.
  - Attention in "scoresT" layout (keys on partitions): causal handled at
    128-column granularity (variable-N matmuls skip masked regions, mask
    added only on diagonal 128x128 blocks). Softmax denominator via an
    M=128 all-ones stationary matmul, which lands the sums broadcast on all
    128 partitions: reciprocal + normalization run full-lane on DVE.
"""
import sys
from contextlib import ExitStack

sys.path.insert(0, "/opt/trn_rl_repo")

import numpy as np
import ml_dtypes

import concourse.bass as bass
import concourse.mybir as mybir
import concourse.tile as tile
from concourse import bacc
from concourse.bass_utils import run_bass_kernel_spmd
from concourse.tile import TileContext

B, S, D = 4, 1024, 4096
H, HD = 32, 128
R = 16
LORA_SCALE = 2.0
N_CORES = 8
HPC = H // N_CORES            # heads per core
FPC = HPC * HD                # features per core = 512
T = B * S                     # 4096 tokens
TT = 256                      # stage-A T-tile (tokens)
NTT = S // TT                 # T-tiles per batch = 4
SCALE = float(1.0 / np.sqrt(HD))
BF = mybir.dt.bfloat16
F32 = mybir.dt.float32


def _bf(a):
    return np.ascontiguousarray(np.asarray(a, np.float32).astype(ml_dtypes.bfloat16))


def _core_perm(c):
    """Column order per core: for each head pair (h0,h1):
    h0-evens(64), h1-evens(64), h0-odds(64), h1-odds(64)."""
    hs = [HPC * c + i for i in range(HPC)]
    ev = np.arange(0, HD, 2)
    od = np.arange(1, HD, 2)
    out = []
    for pair in (0, 1):
        h0, h1 = hs[2 * pair], hs[2 * pair + 1]
        out.append(h0 * HD + ev)
        out.append(h1 * HD + ev)
        out.append(h0 * HD + od)
        out.append(h1 * HD + od)
    return np.concatenate(out)


def _check_causal(mask):
    iu = np.triu_indices(S, k=1)
    il = np.tril_indices(S, k=0)
    if not ((mask[iu] <= -1e8).all() and (mask[il] == 0).all()):
        return False
    # all 128x128 diagonal blocks must be identical
    m0 = mask[0:128, 0:128]
    for j in range(1, S // 128):
        if not np.array_equal(mask[j * 128:(j + 1) * 128,
                                   j * 128:(j + 1) * 128], m0):
            return False
    return True


def _host_prep(x, wq_w, wq_a, wq_b, wk_w, wv_w, wv_a, wv_b, wo_w,
               freqs_cos, freqs_sin, mask):
    x2 = np.asarray(x, np.float32).reshape(T, D)
    xT = _bf(x2.T)

    # fold LoRA (rank-16) into the dense weights on the host
    wq_eff = (np.asarray(wq_w, np.float32)
              + LORA_SCALE * np.asarray(wq_b, np.float32)
              @ np.asarray(wq_a, np.float32))
    wv_eff = (np.asarray(wv_w, np.float32)
              + LORA_SCALE * np.asarray(wv_b, np.float32)
              @ np.asarray(wv_a, np.float32))
    wk_f = np.asarray(wk_w, np.float32)

    cosT = np.asarray(freqs_cos, np.float32).T
    sinT = np.asarray(freqs_sin, np.float32).T
    cc = np.ascontiguousarray(np.tile(cosT, (2, B)).astype(np.float32))
    ss = np.ascontiguousarray(np.tile(sinT, (2, B)).astype(np.float32))

    mask = np.asarray(mask, np.float32)
    causal = _check_causal(mask)
    mT = mask.T * np.float32(np.sqrt(HD))
    if causal:
        # single diagonal block [keys, queries]
        maskp = np.ascontiguousarray(mT[0:128, 0:128])
    else:
        maskp = np.zeros((8, 128, 2, 512), np.float32)
        for qh in range(2):
            for j in range(8):
                maskp[j, :, qh, :] = mT[j * 128:(j + 1) * 128,
                                        qh * 512:(qh + 1) * 512]

    shared = dict(xT=xT, cc=cc, ss=ss, maskp=maskp)
    cores = []
    for c in range(N_CORES):
        perm = _core_perm(c)
        sl = slice(c * FPC, (c + 1) * FPC)
        cores.append(dict(
            wqT=_bf(wq_eff[perm, :].T),
            wkT=_bf(wk_f[perm, :].T),
            wvT=_bf(wv_eff[sl, :].T),
            woT=_bf(np.asarray(wo_w, np.float32)[:, sl].T),
        ))
    return shared, cores, causal


def _blocks(qh, causal):
    """List of (kb, off, n, diag) score blocks for query half qh.
    kb: 128-key block index; off/n: query-column range within the half;
    diag: whether the first 128 columns of the range need the causal mask."""
    out = []
    if causal:
        for kb in range(qh * 4):
            out.append((kb, 0, 512, False))
        for j in range(4):
            kb = qh * 4 + j
            out.append((kb, j * 128, 512 - j * 128, True))
    else:
        for kb in range(8):
            out.append((kb, 0, 512, False))
    return out


def _build_program(causal):
    nc = bacc.Bacc("TRN2", num_devices=N_CORES)

    xT = nc.dram_tensor("xT", [D, T], BF, kind="ExternalInput").ap()
    wqT = nc.dram_tensor("wqT", [D, FPC], BF, kind="ExternalInput").ap()
    wkT = nc.dram_tensor("wkT", [D, FPC], BF, kind="ExternalInput").ap()
    wvT = nc.dram_tensor("wvT", [D, FPC], BF, kind="ExternalInput").ap()
    woT = nc.dram_tensor("woT", [FPC, D], BF, kind="ExternalInput").ap()
    cc = nc.dram_tensor("cc", [128, T], F32, kind="ExternalInput").ap()
    ss = nc.dram_tensor("ss", [128, T], F32, kind="ExternalInput").ap()
    mshape = [128, 128] if causal else [8, 128, 2, 512]
    maskp = nc.dram_tensor("maskp", mshape, F32, kind="ExternalInput").ap()
    y = nc.dram_tensor("y", [T, D], BF, kind="ExternalOutput").ap()

    with TileContext(nc) as tc, ExitStack() as ctx:
        wpool = ctx.enter_context(tc.tile_pool(name="wpool", bufs=1))
        xpool = ctx.enter_context(tc.tile_pool(name="xpool", bufs=2))
        ccp = ctx.enter_context(tc.tile_pool(name="ccp", bufs=4))
        qkvp = ctx.enter_context(tc.tile_pool(name="qkvp", bufs=1))
        tmpp = ctx.enter_context(tc.tile_pool(name="tmpp", bufs=4))
        expp = ctx.enter_context(tc.tile_pool(name="expp", bufs=4))
        recp = ctx.enter_context(tc.tile_pool(name="recp", bufs=2))
        outp = ctx.enter_context(tc.tile_pool(name="outp", bufs=3))
        wop = ctx.enter_context(tc.tile_pool(name="wop", bufs=2))
        pp = ctx.enter_context(tc.tile_pool(name="pp", bufs=2, space="PSUM"))
        pa = ctx.enter_context(tc.tile_pool(name="pa", bufs=4, space="PSUM"))
        psc = ctx.enter_context(tc.tile_pool(name="psc", bufs=2, space="PSUM"))

        # resident weights
        wq_sb = wpool.tile([128, 32, FPC], BF, tag="wq")
        nc.sync.dma_start(wq_sb[:], wqT.rearrange("(o p) f -> p o f", p=128))
        wk_sb = wpool.tile([128, 32, FPC], BF, tag="wk")
        nc.sync.dma_start(wk_sb[:], wkT.rearrange("(o p) f -> p o f", p=128))
        wv_sb = wpool.tile([128, 32, FPC], BF, tag="wv")
        nc.sync.dma_start(wv_sb[:], wvT.rearrange("(o p) f -> p o f", p=128))
        if causal:
            mask_sb = wpool.tile([128, 128], F32, tag="mask")
            nc.sync.dma_start(mask_sb[:], maskp[:])
        else:
            mask_sb = wpool.tile([128, 8, 2, 512], F32, tag="mask")
            nc.sync.dma_start(mask_sb[:],
                              maskp.rearrange("j p q n -> p j q n"))
        ones_sb = wpool.tile([128, 128], BF, tag="ones")
        nc.gpsimd.memset(ones_sb[:], 1.0)

        # per-batch persistent tiles (single-buffered; tile framework
        # serializes next batch's writes behind this batch's readers)
        Qr = qkvp.tile([128, 2, S], BF, tag="Qr")
        Qi = qkvp.tile([128, 2, S], BF, tag="Qi")
        Kr = qkvp.tile([128, 2, S], BF, tag="Kr")
        Ki = qkvp.tile([128, 2, S], BF, tag="Ki")
        V_sb = qkvp.tile([128, 8, FPC], BF, tag="Vsb")
        OT_sb = qkvp.tile([128, 4, S], BF, tag="OT")

        tiles = [(b, tt) for b in range(B) for tt in range(NTT)]
        xts, ccts,ssts = {}, {}, {}

        def load_tile(i):
            b, tt = tiles[i]
            t0 = b * S + tt * TT
            x_sb = xpool.tile([128, 32, TT], BF, tag="x")
            nc.sync.dma_start(
                x_sb[:],
                xT.rearrange("(o p) t -> p o t", p=128)[:, :, t0:t0 + TT])
            cc_sb = ccp.tile([128, TT], F32, tag="cc")
            nc.sync.dma_start(cc_sb[:], cc[:, t0:t0 + TT])
            ss_sb = ccp.tile([128, TT], F32, tag="ss")
            nc.sync.dma_start(ss_sb[:], ss[:, t0:t0 + TT])
            xts[i], ccts[i], ssts[i] = x_sb, cc_sb, ss_sb

        def stage_a(i):
            b, tt = tiles[i]
            x_sb, cc_sb, ss_sb = xts.pop(i), ccts.pop(i), ssts.pop(i)
            toff = tt * TT
            for dst_r, dst_i, w_sb in ((Qr, Qi, wq_sb), (Kr, Ki, wk_sb)):
                for pair in range(2):
                    f0 = pair * 256
                    ps_r = pp.tile([128, 512], F32, tag="pp")
                    for d in range(32):
                        nc.tensor.matmul(
                            ps_r[:, 0:TT], w_sb[:, d, f0:f0 + 128],
                            x_sb[:, d, :], start=(d == 0), stop=(d == 31))
                    ps_i = pp.tile([128, 512], F32, tag="pp")
                    for d in range(32):
                        nc.tensor.matmul(
                            ps_i[:, 0:TT], w_sb[:, d, f0 + 128:f0 + 256],
                            x_sb[:, d, :], start=(d == 0), stop=(d == 31))
                    t1 = tmpp.tile([128, TT], BF, tag="t")
                    nc.vector.tensor_mul(t1[:], ps_r[:, 0:TT], cc_sb[:])
                    t2 = tmpp.tile([128, TT], BF, tag="t")
                    nc.vector.tensor_mul(t2[:], ps_i[:, 0:TT], ss_sb[:])
                    nc.vector.tensor_tensor(
                        dst_r[:, pair, toff:toff + TT], t1[:], t2[:],
                        mybir.AluOpType.subtract)
                    t3 = tmpp.tile([128, TT], BF, tag="t")
                    nc.vector.tensor_mul(t3[:], ps_r[:, 0:TT], ss_sb[:])
                    t4 = tmpp.tile([128, TT], BF, tag="t")
                    nc.vector.tensor_mul(t4[:], ps_i[:, 0:TT], cc_sb[:])
                    nc.vector.tensor_tensor(
                        dst_i[:, pair, toff:toff + TT], t3[:], t4[:],
                        mybir.AluOpType.add)
            # V natural: per 128-token block
            for v in range(TT // 128):
                tb = tt * (TT // 128) + v
                ps_v = pp.tile([128, 512], F32, tag="pp")
                for d in range(32):
                    nc.tensor.matmul(
                        ps_v[:], x_sb[:, d, v * 128:(v + 1) * 128],
                        wv_sb[:, d, :], start=(d == 0), stop=(d == 31))
                nc.scalar.copy(V_sb[:, tb, :], ps_v[:])

        def attn_half(qh):
            q0 = qh * 512
            blocks = _blocks(qh, causal)
            nblk = len(blocks)
            for pair in range(2):
                ps_ot = [pa.tile([128, 512], F32, tag="pa", name=f"ot{h}")
                         for h in range(2)]
                ps_sum = [pa.tile([128, 512], F32, tag="pa", name=f"sum{h}")
                          for h in range(2)]

                def pv_sum(j, es):
                    # denominator + PV matmuls for block j (PE consumers of
                    # exp output; emitted one block late so the PE has a
                    # scores round in flight while Scalar finishes exp)
                    kb, off, n, diag = blocks[j]
                    first, last = (j == 0), (j == nblk - 1)
                    for h in range(2):
                        l = 2 * pair + h
                        nc.tensor.matmul(
                            ps_sum[h][:, off:off + n], ones_sb[:],
                            es[h][:, off:off + n], start=first, stop=last)
                        nc.tensor.matmul(
                            ps_ot[h][:, off:off + n],
                            V_sb[:, kb, l * 128:(l + 1) * 128],
                            es[h][:, off:off + n], start=first, stop=last)

                prev = None
                for j, (kb, off, n, diag) in enumerate(blocks):
                    k0 = kb * 128
                    # scores: r and i contributions as K=64 row tiles,
                    # heads of the pair interleaved so the PE can overlap
                    # the disjoint row halves; sc banks alternate between
                    # two psum pools so two blocks can be in flight
                    # sc banks alternate between the psc pool and the (idle
                    # during attention) stage-A pp pool so two blocks can be
                    # in flight; tag must match the pool's existing ring
                    scpool, sctag = (psc, "sc") if j % 2 == 0 else (pp, "pp")
                    sc = [scpool.tile([128, 512], F32, tag=sctag,
                                      name=f"sc{h}")
                          for h in range(2)]
                    for src_q, src_k in ((Qr, Kr), (Qi, Ki)):
                        for h in range(2):
                            bp = h * 64
                            nc.tensor.matmul(
                                sc[h][:, off:off + n],
                                src_k[bp:bp + 64, pair, k0:k0 + 128],
                                src_q[bp:bp + 64, pair,
                                      q0 + off:q0 + off + n],
                                start=(src_q is Qr), stop=(src_q is Qi))
                    es = [None, None]
                    for h in range(2):
                        if causal:
                            if diag:
                                nc.vector.tensor_add(
                                    sc[h][:, off:off + 128],
                                    sc[h][:, off:off + 128], mask_sb[:])
                        else:
                            nc.vector.tensor_add(
                                sc[h][:, off:off + n], sc[h][:, off:off + n],
                                mask_sb[:, kb, qh, off:off + n])
                        es[h] = expp.tile([128, 512], BF, tag="e",
                                          name=f"e{h}")
                        nc.scalar.activation(
                            es[h][:, off:off + n], sc[h][:, off:off + n],
                            mybir.ActivationFunctionType.Exp, scale=SCALE)
                    if prev is not None:
                        pv_sum(*prev)
                    prev = (j, es)
                pv_sum(*prev)
                for h in range(2):
                    l = 2 * pair + h
                    rec = recp.tile([128, 512], F32, tag="rec")
                    # sums are positive and well away from denorm/inf, so the
                    # fast ~18-bit approximation is plenty for normalization
                    nc.vector.reciprocal_approx_fast(rec[:], ps_sum[h][:])
                    nc.vector.tensor_mul(OT_sb[:, l, q0:q0 + 512],
                                         ps_ot[h][:], rec[:])

        wots = {}

        def load_wo(nt):
            wo_sb = wop.tile([128, 4, 512], BF, tag="wo")
            nc.sync.dma_start(
                wo_sb[:],
                woT.rearrange("(o p) n -> p o n",
                              p=128)[:, :, nt * 512:(nt + 1) * 512])
            wots[nt] = wo_sb

        load_tile(0)
        for i, (b, tt) in enumerate(tiles):
            if i + 1 < len(tiles):
                load_tile(i + 1)
            stage_a(i)
            if tt == 1:
                attn_half(0)
            if tt == 3:
                load_wo(0)
                load_wo(1)
                attn_half(1)
                # stage C; wo prefetched 2-ahead AFTER this nt's readers
                # are emitted (so the buf-reuse WAR is tracked correctly)
                for nt in range(8):
                    wo_sb = wots.pop(nt)
                    for tb in range(8):
                        ps_o = pa.tile([128, 512], F32, tag="pa")
                        for k in range(4):
                            nc.tensor.matmul(
                                ps_o[:],
                                OT_sb[:, k, tb * 128:(tb + 1) * 128],
                                wo_sb[:, k, :], start=(k == 0), stop=(k == 3))
                        o_sb = outp.tile([128, 512], BF, tag="o")
                        nc.scalar.copy(o_sb[:], ps_o[:])
                        nc.sync.dma_start(
                            y[b * S + tb * 128:b * S + (tb + 1) * 128,
                              nt * 512:(nt + 1) * 512], o_sb[:])
                    if nt + 2 < 8:
                        load_wo(nt + 2)

    nc.compile()
    return nc


_CACHE = {}


def _get_program(causal):
    if causal not in _CACHE:
        _CACHE[causal] = _build_program(causal)
    return _CACHE[causal]


def kernel(x, wq_w, wq_a, wq_b, wk_w, wv_w, wv_a, wv_b, wo_w,
           freqs_cos, freqs_sin, mask, start_pos=0, _trace=False):
    assert int(np.asarray(start_pos)) == 0
    shared, cores, causal = _host_prep(
        x, wq_w, wq_a, wq_b, wk_w, wv_w, wv_a, wv_b, wo_w,
        freqs_cos, freqs_sin, mask)
    nc = _get_program(causal)
    in_maps = []
    for c in range(N_CORES):
        m = dict(xT=shared["xT"], cc=shared["cc"], ss=shared["ss"],
                 maskp=shared["maskp"])
        m.update(cores[c])
        in_maps.append(m)
    res = run_bass_kernel_spmd(nc, in_maps, list(range(N_CORES)),
                               trace=_trace)
    kernel._last_results = res
    acc = np.zeros((T, D), np.float32)
    for c in range(N_CORES):
        acc += np.asarray(res.results[c]["y"], np.float32)
    out = acc.reshape(B, S, D)
    return out.astype(np.asarray(x).dtype, copy=False)


# revision 12
# speedup vs baseline: 1.4430x; 1.1295x over previous
"""Trainium2 Bass kernel for LoRA attention prefill (B=4, S=1024, D=4096, H=32).

Sharding: tensor-parallel over heads. Each of the 8 cores computes 4 heads
(512 of the 4096 q/k/v features, column-shard of wq/wk/wv) and a row-shard
of wo, producing a full-shape [T, D] partial output in bf16; partials are
summed on the host.

Device layout choices:
  - LoRA folded into wq/wv on the host (rank-16 update), so the device only
    runs dense projections.
  - All matmuls in bf16 with fp32 PSUM accumulation.
  - Activations kept feature-on-partition: xT [D, T] feeds Q/K projections
    directly as PE operands; V is computed token-on-partition so it serves
    as the PV-matmul stationary operand without transposes.
  - RoPE: wq/wk rows host-permuted so each head pair's even (real) and odd
    (imag) feature halves come out of separate psum chains lane-aligned with
    cos/sin tiles; the rotated results are stored in separate Qr/Qi (Kr/Ki)
    tiles with NO partition regrouping. Scores for head h contract as two
    K=64 matmuls (real + imag) on disjoint PE row tiles; the two heads of a
    pair are interleaved so the row tiles can overlap on the array.
  - Attention in "scoresT" layout (keys on partitions): causal handled at
    128-column granularity (variable-N matmuls skip masked regions, mask
    added only on diagonal 128x128 blocks). Softmax denominator via an
    M=128 all-ones stationary matmul, which lands the sums broadcast on all
    128 partitions: reciprocal + normalization run full-lane on DVE.
"""
import sys
from contextlib import ExitStack

sys.path.insert(0, "/opt/trn_rl_repo")

import numpy as np
import ml_dtypes

import concourse.bass as bass
import concourse.mybir as mybir
import concourse.tile as tile
from concourse import bacc
from concourse.bass_utils import run_bass_kernel_spmd
from concourse.tile import TileContext

B, S, D = 4, 1024, 4096
H, HD = 32, 128
R = 16
LORA_SCALE = 2.0
N_CORES = 8
HPC = H // N_CORES            # heads per core
FPC = HPC * HD                # features per core = 512
T = B * S                     # 4096 tokens
TT = 256                      # stage-A T-tile (tokens)
NTT = S // TT                 # T-tiles per batch = 4
SCALE = float(1.0 / np.sqrt(HD))
BF = mybir.dt.bfloat16
F32 = mybir.dt.float32


def _bf(a):
    return np.ascontiguousarray(np.asarray(a, np.float32).astype(ml_dtypes.bfloat16))


def _core_perm(c):
    """Column order per core: for each head pair (h0,h1):
    h0-evens(64), h1-evens(64), h0-odds(64), h1-odds(64)."""
    hs = [HPC * c + i for i in range(HPC)]
    ev = np.arange(0, HD, 2)
    od = np.arange(1, HD, 2)
    out = []
    for pair in (0, 1):
        h0, h1 = hs[2 * pair], hs[2 * pair + 1]
        out.append(h0 * HD + ev)
        out.append(h1 * HD + ev)
        out.append(h0 * HD + od)
        out.append(h1 * HD + od)
    return np.concatenate(out)


def _check_causal(mask):
    iu = np.triu_indices(S, k=1)
    il = np.tril_indices(S, k=0)
    if not ((mask[iu] <= -1e8).all() and (mask[il] == 0).all()):
        return False
    # all 128x128 diagonal blocks must be identical
    m0 = mask[0:128, 0:128]
    for j in range(1, S // 128):
        if not np.array_equal(mask[j * 128:(j + 1) * 128,
                                   j * 128:(j + 1) * 128], m0):
            return False
    return True


def _host_prep(x, wq_w, wq_a, wq_b, wk_w, wv_w, wv_a, wv_b, wo_w,
               freqs_cos, freqs_sin, mask):
    x2 = np.asarray(x, np.float32).reshape(T, D)
    xT = _bf(x2.T)

    # fold LoRA (rank-16) into the dense weights on the host
    wq_eff = (np.asarray(wq_w, np.float32)
              + LORA_SCALE * np.asarray(wq_b, np.float32)
              @ np.asarray(wq_a, np.float32))
    wv_eff = (np.asarray(wv_w, np.float32)
              + LORA_SCALE * np.asarray(wv_b, np.float32)
              @ np.asarray(wv_a, np.float32))
    wk_f = np.asarray(wk_w, np.float32)

    cosT = np.asarray(freqs_cos, np.float32).T
    sinT = np.asarray(freqs_sin, np.float32).T
    cc = np.ascontiguousarray(np.tile(cosT, (2, B)).astype(np.float32))
    ss = np.ascontiguousarray(np.tile(sinT, (2, B)).astype(np.float32))

    mask = np.asarray(mask, np.float32)
    causal = _check_causal(mask)
    mT = mask.T * np.float32(np.sqrt(HD))
    if causal:
        # single diagonal block [keys, queries]
        maskp = np.ascontiguousarray(mT[0:128, 0:128])
    else:
        maskp = np.zeros((8, 128, 2, 512), np.float32)
        for qh in range(2):
            for j in range(8):
                maskp[j, :, qh, :] = mT[j * 128:(j + 1) * 128,
                                        qh * 512:(qh + 1) * 512]

    shared = dict(xT=xT, cc=cc, ss=ss, maskp=maskp)
    cores = []
    for c in range(N_CORES):
        perm = _core_perm(c)
        sl = slice(c * FPC, (c + 1) * FPC)
        cores.append(dict(
            wqT=_bf(wq_eff[perm, :].T),
            wkT=_bf(wk_f[perm, :].T),
            wvT=_bf(wv_eff[sl, :].T),
            woT=_bf(np.asarray(wo_w, np.float32)[:, sl].T),
        ))
    return shared, cores, causal


def _blocks(qh, causal):
    """List of (kb, off, n, diag) score blocks for query half qh.
    kb: 128-key block index; off/n: query-column range within the half;
    diag: whether the first 128 columns of the range need the causal mask."""
    out = []
    if causal:
        for kb in range(qh * 4):
            out.append((kb, 0, 512, False))
        for j in range(4):
            kb = qh * 4 + j
            out.append((kb, j * 128, 512 - j * 128, True))
    else:
        for kb in range(8):
            out.append((kb, 0, 512, False))
    return out


def _build_program(causal):
    nc = bacc.Bacc("TRN2", num_devices=N_CORES)

    xT = nc.dram_tensor("xT", [D, T], BF, kind="ExternalInput").ap()
    wqT = nc.dram_tensor("wqT", [D, FPC], BF, kind="ExternalInput").ap()
    wkT = nc.dram_tensor("wkT", [D, FPC], BF, kind="ExternalInput").ap()
    wvT = nc.dram_tensor("wvT", [D, FPC], BF, kind="ExternalInput").ap()
    woT = nc.dram_tensor("woT", [FPC, D], BF, kind="ExternalInput").ap()
    cc = nc.dram_tensor("cc", [128, T], F32, kind="ExternalInput").ap()
    ss = nc.dram_tensor("ss", [128, T], F32, kind="ExternalInput").ap()
    mshape = [128, 128] if causal else [8, 128, 2, 512]
    maskp = nc.dram_tensor("maskp", mshape, F32, kind="ExternalInput").ap()
    y = nc.dram_tensor("y", [T, D], BF, kind="ExternalOutput").ap()

    with TileContext(nc) as tc, ExitStack() as ctx:
        wpool = ctx.enter_context(tc.tile_pool(name="wpool", bufs=1))
        xpool = ctx.enter_context(tc.tile_pool(name="xpool", bufs=2))
        ccp = ctx.enter_context(tc.tile_pool(name="ccp", bufs=4))
        qkvp = ctx.enter_context(tc.tile_pool(name="qkvp", bufs=1))
        tmpp = ctx.enter_context(tc.tile_pool(name="tmpp", bufs=4))
        expp = ctx.enter_context(tc.tile_pool(name="expp", bufs=4))
        recp = ctx.enter_context(tc.tile_pool(name="recp", bufs=2))
        outp = ctx.enter_context(tc.tile_pool(name="outp", bufs=2))
        wop = ctx.enter_context(tc.tile_pool(name="wop", bufs=2))
        pp = ctx.enter_context(tc.tile_pool(name="pp", bufs=2, space="PSUM"))
        pa = ctx.enter_context(tc.tile_pool(name="pa", bufs=4, space="PSUM"))
        psc = ctx.enter_context(tc.tile_pool(name="psc", bufs=2, space="PSUM"))

        # resident weights, split per head pair so the first Q chains can
        # start after ~3MB of DMA instead of 12MB (startup PE gap)
        wq_sb, wk_sb = [], []
        wv_sb = wpool.tile([128, 32, FPC], BF, tag="wv")
        for pair in range(2):
            t = wpool.tile([128, 32, 256], BF, tag=f"wq{pair}",
                           name=f"wq{pair}")
            wq_sb.append(t)
        for pair in range(2):
            t = wpool.tile([128, 32, 256], BF, tag=f"wk{pair}",
                           name=f"wk{pair}")
            wk_sb.append(t)

        def load_weights():
            for pair in range(2):
                nc.sync.dma_start(
                    wq_sb[pair][:],
                    wqT.rearrange("(o p) f -> p o f",
                                  p=128)[:, :, pair * 256:(pair + 1) * 256])
            for pair in range(2):
                nc.sync.dma_start(
                    wk_sb[pair][:],
                    wkT.rearrange("(o p) f -> p o f",
                                  p=128)[:, :, pair * 256:(pair + 1) * 256])
            nc.sync.dma_start(wv_sb[:],
                              wvT.rearrange("(o p) f -> p o f", p=128))
        if causal:
            mask_sb = wpool.tile([128, 128], F32, tag="mask")
        else:
            mask_sb = wpool.tile([128, 8, 2, 512], F32, tag="mask")
        ones_sb = wpool.tile([128, 128], BF, tag="ones")

        def load_mask_ones():
            if causal:
                nc.sync.dma_start(mask_sb[:], maskp[:])
            else:
                nc.sync.dma_start(mask_sb[:],
                                  maskp.rearrange("j p q n -> p j q n"))
            nc.gpsimd.memset(ones_sb[:], 1.0)

        # per-batch persistent tiles (single-buffered; tile framework
        # serializes next batch's writes behind this batch's readers)
        Qr = qkvp.tile([128, 2, S], BF, tag="Qr")
        Qi = qkvp.tile([128, 2, S], BF, tag="Qi")
        Kr = qkvp.tile([128, 2, S], BF, tag="Kr")
        Ki = qkvp.tile([128, 2, S], BF, tag="Ki")
        V_sb = qkvp.tile([128, 8, FPC], BF, tag="Vsb")
        # one OT tile per query half so stage C's first half doesn't RAW-wait
        # (whole-tile dep tracking) on the second half's normalization
        OT0 = qkvp.tile([128, 4, 512], BF, tag="OT0")
        OT1 = qkvp.tile([128, 4, 512], BF, tag="OT1")

        tiles = [(b, tt) for b in range(B) for tt in range(NTT)]
        xts, ccts,ssts = {}, {}, {}

        def load_tile(i):
            b, tt = tiles[i]
            t0 = b * S + tt * TT
            x_sb = xpool.tile([128, 32, TT], BF, tag="x")
            nc.sync.dma_start(
                x_sb[:],
                xT.rearrange("(o p) t -> p o t", p=128)[:, :, t0:t0 + TT])
            cc_sb = ccp.tile([128, TT], F32, tag="cc")
            nc.sync.dma_start(cc_sb[:], cc[:, t0:t0 + TT])
            ss_sb = ccp.tile([128, TT], F32, tag="ss")
            nc.sync.dma_start(ss_sb[:], ss[:, t0:t0 + TT])
            xts[i], ccts[i], ssts[i] = x_sb, cc_sb, ss_sb

        def stage_a(i):
            b, tt = tiles[i]
            x_sb, cc_sb, ss_sb = xts.pop(i), ccts.pop(i), ssts.pop(i)
            toff = tt * TT
            for dst_r, dst_i, w_sb in ((Qr, Qi, wq_sb), (Kr, Ki, wk_sb)):
                for pair in range(2):
                    wp = w_sb[pair]
                    ps_r = pp.tile([128, 512], F32, tag="pp")
                    for d in range(32):
                        nc.tensor.matmul(
                            ps_r[:, 0:TT], wp[:, d, 0:128],
                            x_sb[:, d, :], start=(d == 0), stop=(d == 31))
                    ps_i = pp.tile([128, 512], F32, tag="pp")
                    for d in range(32):
                        nc.tensor.matmul(
                            ps_i[:, 0:TT], wp[:, d, 128:256],
                            x_sb[:, d, :], start=(d == 0), stop=(d == 31))
                    # both ps_r readers (t1, t3) are emitted FIRST so the DVE
                    # FIFO frees the ps_r bank while the ps_i chain is still
                    # on the PE; otherwise the next pair's chain WAR-stalls
                    # ~0.85us on every pair
                    t1 = tmpp.tile([128, TT], BF, tag="t")
                    nc.vector.tensor_mul(t1[:], ps_r[:, 0:TT], cc_sb[:])
                    t3 = tmpp.tile([128, TT], BF, tag="t")
                    nc.vector.tensor_mul(t3[:], ps_r[:, 0:TT], ss_sb[:])
                    t2 = tmpp.tile([128, TT], BF, tag="t")
                    nc.vector.tensor_mul(t2[:], ps_i[:, 0:TT], ss_sb[:])
                    nc.vector.tensor_tensor(
                        dst_r[:, pair, toff:toff + TT], t1[:], t2[:],
                        mybir.AluOpType.subtract)
                    t4 = tmpp.tile([128, TT], BF, tag="t")
                    nc.vector.tensor_mul(t4[:], ps_i[:, 0:TT], cc_sb[:])
                    nc.vector.tensor_tensor(
                        dst_i[:, pair, toff:toff + TT], t3[:], t4[:],
                        mybir.AluOpType.add)
            # V natural: per 128-token block
            for v in range(TT // 128):
                tb = tt * (TT // 128) + v
                ps_v = pp.tile([128, 512], F32, tag="pp")
                for d in range(32):
                    nc.tensor.matmul(
                        ps_v[:], x_sb[:, d, v * 128:(v + 1) * 128],
                        wv_sb[:, d, :], start=(d == 0), stop=(d == 31))
                nc.scalar.copy(V_sb[:, tb, :], ps_v[:])

        def attn_half(qh):
            q0 = qh * 512
            blocks = _blocks(qh, causal)
            nblk = len(blocks)
            for pair in range(2):
                ps_ot = [pa.tile([128, 512], F32, tag="pa", name=f"ot{h}")
                         for h in range(2)]
                ps_sum = [pa.tile([128, 512], F32, tag="pa", name=f"sum{h}")
                          for h in range(2)]

                def pv_sum(j, es):
                    # denominator + PV matmuls for block j (PE consumers of
                    # exp output; emitted one block late so the PE has a
                    # scores round in flight while Scalar finishes exp)
                    kb, off, n, diag = blocks[j]
                    first, last = (j == 0), (j == nblk - 1)
                    for h in range(2):
                        l = 2 * pair + h
                        nc.tensor.matmul(
                            ps_sum[h][:, off:off + n], ones_sb[:],
                            es[h][:, off:off + n], start=first, stop=last)
                        nc.tensor.matmul(
                            ps_ot[h][:, off:off + n],
                            V_sb[:, kb, l * 128:(l + 1) * 128],
                            es[h][:, off:off + n], start=first, stop=last)

                prev = None
                for j, (kb, off, n, diag) in enumerate(blocks):
                    k0 = kb * 128
                    # scores: r and i contributions as K=64 row tiles,
                    # heads of the pair interleaved so the PE can overlap
                    # the disjoint row halves; sc banks alternate between
                    # two psum pools so two blocks can be in flight
                    # sc banks alternate between the psc pool and the (idle
                    # during attention) stage-A pp pool so two blocks can be
                    # in flight; tag must match the pool's existing ring
                    scpool, sctag = (psc, "sc") if j % 2 == 0 else (pp, "pp")
                    sc = [scpool.tile([128, 512], F32, tag=sctag,
                                      name=f"sc{h}")
                          for h in range(2)]
                    for src_q, src_k in ((Qr, Kr), (Qi, Ki)):
                        for h in range(2):
                            bp = h * 64
                            nc.tensor.matmul(
                                sc[h][:, off:off + n],
                                src_k[bp:bp + 64, pair, k0:k0 + 128],
                                src_q[bp:bp + 64, pair,
                                      q0 + off:q0 + off + n],
                                start=(src_q is Qr), stop=(src_q is Qi))
                    es = [None, None]
                    for h in range(2):
                        if causal:
                            if diag:
                                nc.vector.tensor_add(
                                    sc[h][:, off:off + 128],
                                    sc[h][:, off:off + 128], mask_sb[:])
                        else:
                            nc.vector.tensor_add(
                                sc[h][:, off:off + n], sc[h][:, off:off + n],
                                mask_sb[:, kb, qh, off:off + n])
                        es[h] = expp.tile([128, 512], BF, tag="e",
                                          name=f"e{h}")
                        nc.scalar.activation(
                            es[h][:, off:off + n], sc[h][:, off:off + n],
                            mybir.ActivationFunctionType.Exp, scale=SCALE)
                    if prev is not None:
                        pv_sum(*prev)
                    prev = (j, es)
                pv_sum(*prev)
                for h in range(2):
                    l = 2 * pair + h
                    rec = recp.tile([128, 512], F32, tag="rec")
                    # sums are positive and well away from denorm/inf, so the
                    # fast ~18-bit approximation is plenty for normalization
                    nc.vector.reciprocal_approx_fast(rec[:], ps_sum[h][:])
                    OT = OT0 if qh == 0 else OT1
                    nc.vector.tensor_mul(OT[:, l, :], ps_ot[h][:], rec[:])

        wots = {}

        def load_wo(nt):
            wo_sb = wop.tile([128, 4, 512], BF, tag="wo")
            nc.sync.dma_start(
                wo_sb[:],
                woT.rearrange("(o p) n -> p o n",
                              p=128)[:, :, nt * 512:(nt + 1) * 512])
            wots[nt] = wo_sb

        load_tile(0)
        load_weights()
        load_mask_ones()
        for i, (b, tt) in enumerate(tiles):
            if i + 1 < len(tiles):
                load_tile(i + 1)
            stage_a(i)
            if tt == 1:
                attn_half(0)
            if tt == 3:
                load_wo(0)
                load_wo(1)
                attn_half(1)
                # stage C; wo prefetched 2-ahead AFTER this nt's readers
                # are emitted (so the buf-reuse WAR is tracked correctly)
                for nt in range(8):
                    wo_sb = wots.pop(nt)
                    o_sb = outp.tile([128, 8, 512], BF, tag="o")
                    for tb in range(8):
                        OT = OT0 if tb < 4 else OT1
                        tbh = tb % 4
                        ps_o = pa.tile([128, 512], F32, tag="pa")
                        for k in range(4):
                            nc.tensor.matmul(
                                ps_o[:],
                                OT[:, k, tbh * 128:(tbh + 1) * 128],
                                wo_sb[:, k, :], start=(k == 0), stop=(k == 3))
                        nc.scalar.copy(o_sb[:, tb, :], ps_o[:])
                    # one strided store per nt instead of 8: the Sync
                    # engine's ~1us per-DMA cost was throttling stage C
                    nc.sync.dma_start(
                        y[b * S:(b + 1) * S,
                          nt * 512:(nt + 1) * 512].rearrange(
                              "(tb p) n -> p tb n", p=128), o_sb[:])
                    if nt + 2 < 8:
                        load_wo(nt + 2)

    nc.compile()
    return nc


_CACHE = {}


def _get_program(causal):
    if causal not in _CACHE:
        _CACHE[causal] = _build_program(causal)
    return _CACHE[causal]


def kernel(x, wq_w, wq_a, wq_b, wk_w, wv_w, wv_a, wv_b, wo_w,
           freqs_cos, freqs_sin, mask, start_pos=0, _trace=False):
    assert int(np.asarray(start_pos)) == 0
    shared, cores, causal = _host_prep(
        x, wq_w, wq_a, wq_b, wk_w, wv_w, wv_a, wv_b, wo_w,
        freqs_cos, freqs_sin, mask)
    nc = _get_program(causal)
    in_maps = []
    for c in range(N_CORES):
        m = dict(xT=shared["xT"], cc=shared["cc"], ss=shared["ss"],
                 maskp=shared["maskp"])
        m.update(cores[c])
        in_maps.append(m)
    res = run_bass_kernel_spmd(nc, in_maps, list(range(N_CORES)),
                               trace=_trace)
    kernel._last_results = res
    acc = np.zeros((T, D), np.float32)
    for c in range(N_CORES):
        acc += np.asarray(res.results[c]["y"], np.float32)
    out = acc.reshape(B, S, D)
    return out.astype(np.asarray(x).dtype, copy=False)
